# revision 10
# baseline (speedup 1.0000x reference)
"""Categorical-diffusion grayscale sampling kernel for Trainium2 (8 NeuronCores).

Reference math (see the oracle):
    abar = prod(1 - beta[:t])
    bar_Q = abar*I + (1-abar)/K * ones          (K x K, symmetric)
    p     = x0 @ bar_Q = abar*x0 + (1-abar)/K * rowsum(x0)
    out   = softmax(log(p) + g)                 g = -log(-log(u)), u = uniform(key(1))
          = p*E / rowsum(p*E)                   E = exp(g)  (input-independent constant)

The softmax output is invariant to any per-row rescale of p, so divide p by
c = (1-abar)/K:
    P' = ra*x0 + rowsum(x0),   ra = K*abar/(1-abar)
    out = (P'*E) / rowsum(P'*E)

This removes the KxK matmul entirely: the device does one free-axis reduce,
one fused scale-add (ScalarE), one fused multiply+row-reduce (VectorE), a
reciprocal and one fused scale (ScalarE) per tile -- purely memory bound.

Sharding: pure data parallelism over the batch axis (rows of the flattened
[B*H*W, K] view) across 8 cores; beta is replicated; abar/ra are computed
on-device from beta.
"""

import numpy as np

N_CORES = 8
P = 128  # SBUF partitions


# ----------------------------------------------------------------------------
# Gumbel noise constant: E = exp(-log(-log(u))), u = uniform(key(1), shape).
# This is independent of every kernel input (the reference hardcodes key(1)),
# so it is a compile-time constant of the problem; we materialize it on host
# once and stream it through the kernel like any other operand.
# ----------------------------------------------------------------------------
_NOISE_CACHE: dict = {}


def _gumbel_exp(shape) -> np.ndarray:
    key_shape = tuple(shape)
    e = _NOISE_CACHE.get(key_shape)
    if e is None:
        import jax
        import jax.numpy as jnp

        cpu = jax.devices("cpu")[0]
        with jax.default_device(cpu):
            u = jax.random.uniform(
                jax.random.key(1), key_shape, jnp.float32, minval=1e-9, maxval=1.0
            )
            gumbel = -jnp.log(-jnp.log(u))
            e = np.asarray(jnp.exp(gumbel), dtype=np.float32)
        _NOISE_CACHE[key_shape] = e
    return e


# ----------------------------------------------------------------------------
# Bass kernel builder
# ----------------------------------------------------------------------------
_NC_CACHE: dict = {}

# Set by test harness for profiling; kernel() publishes the last run's
# HW exec time here when TRACE is enabled.
TRACE = False
LAST_EXEC_NS = None
LAST_RESULTS = None


def _build_nc(rows_pc: int, K: int, t: int, G: int, x_dt: str, onehot: bool):
    """Build + compile the per-core Bass kernel.

    rows_pc: rows per core (flattened b*h*w), must be divisible by P*G.
    K: category count (256).
    t: timestep (length of the beta prefix). t == 0 means p == x0.
    G: row-tiles per DMA group.
    x_dt: dtype x is streamed as ("f32"/"bf16"/"fp8"; the host only picks a
          narrow type when the cast round-trips exactly, e.g. one-hot 0/1).
    onehot: host verified every row of x is exactly one-hot, so rowsum(x)
        is exactly 1.0 in any summation order -- skip the on-device reduce.
    """
    from contextlib import ExitStack

    import concourse.bass as bass  # noqa: F401
    import concourse.tile as tile
    from concourse import bacc, mybir

    f32 = mybir.dt.float32
    dt_x = {"f32": f32, "bf16": mybir.dt.bfloat16, "fp8": mybir.dt.float8e4}[x_dt]
    AX = mybir.AxisListType
    ALU = mybir.AluOpType
    ACT = mybir.ActivationFunctionType

    n_tiles = rows_pc // P
    assert n_tiles % G == 0, (rows_pc, G)
    n_groups = n_tiles // G

    nc = bacc.Bacc("TRN2", target_bir_lowering=False, debug=False)

    x_d = nc.dram_tensor("x", [rows_pc, K], dt_x, kind="ExternalInput").ap()
    e_d = nc.dram_tensor("e", [rows_pc, K], f32, kind="ExternalInput").ap()
    if t > 0:
        b_d = nc.dram_tensor("beta", [t], f32, kind="ExternalInput").ap()
    o_d = nc.dram_tensor("out", [rows_pc, K], f32, kind="ExternalOutput").ap()

    # p-major row layout: partition p owns rows [p*n_tiles, (p+1)*n_tiles).
    # Every op is row-independent, so only the (host-transparent) row->(p,g,j)
    # mapping changes; each partition's DMA slice is one contiguous G*K chunk.
    xv = x_d.rearrange("(p g j) k -> g p j k", p=P, j=G)
    ev = e_d.rearrange("(p g j) k -> g p j k", p=P, j=G)
    ov = o_d.rearrange("(p g j) k -> g p j k", p=P, j=G)

    with tile.TileContext(nc) as tc, ExitStack() as ctx:
        ra_t = None
        if t > 0:
            # Preamble: ra = K*abar/(1-abar), abar = exp(sum(ln(1-beta[:t]))).
            # Computed redundantly on all 128 partitions (beta broadcast) so the
            # result is directly usable as a per-partition activation scale.
            cpool = ctx.enter_context(tc.tile_pool(name="cpool", bufs=1))
            bt = cpool.tile([P, t], f32)
            nc.sync.dma_start(bt[:], b_d.unsqueeze(0).broadcast_to([P, t]))
            lnb = cpool.tile([P, t], f32)
            lnacc = cpool.tile([P, 1], f32)
            # lnb = Ln(-beta + 1), lnacc = rowsum(lnb) = ln(abar)
            nc.scalar.activation(
                lnb[:], bt[:], ACT.Ln, bias=1.0, scale=-1.0, accum_out=lnacc[:]
            )
            a_t = cpool.tile([P, 1], f32)
            nc.scalar.activation(a_t[:], lnacc[:], ACT.Exp)  # abar
            d_t = cpool.tile([P, 1], f32)
            nc.vector.tensor_scalar(d_t[:], a_t[:], -1.0, 1.0, ALU.mult, ALU.add)
            rd_t = cpool.tile([P, 1], f32)
            nc.vector.reciprocal(rd_t[:], d_t[:])  # 1/(1-abar)
            ka_t = cpool.tile([P, 1], f32)
            nc.vector.tensor_scalar_mul(ka_t[:], a_t[:], float(K))  # K*abar
            ra_t = cpool.tile([P, 1], f32)
            nc.vector.tensor_mul(ra_t[:], ka_t[:], rd_t[:])  # ra

        # Adaptive buffering: bump pools from triple- to quad-buffering in
        # order of usefulness (e loads, o stores, x loads) while the working
        # set stays under the SBUF budget.
        kb = 1024
        szx = G * K * mybir.dt.size(dt_x)
        szf = G * K * 4
        bufs = {"x": 3, "e": 3, "p": 2, "o": 3}
        size = {"x": szx, "e": szf, "p": szf, "o": szf}
        budget = 186 * kb - (t * 4 * 2 if t > 0 else 0)  # minus cpool
        used = sum(bufs[k] * size[k] for k in bufs)
        for k in ("e", "o", "x"):
            if used + size[k] <= budget:
                bufs[k] += 1
                used += size[k]
        x_pool = ctx.enter_context(tc.tile_pool(name="x_pool", bufs=bufs["x"]))
        e_pool = ctx.enter_context(tc.tile_pool(name="e_pool", bufs=bufs["e"]))
        p_pool = ctx.enter_context(tc.tile_pool(name="p_pool", bufs=bufs["p"]))
        o_pool = ctx.enter_context(tc.tile_pool(name="o_pool", bufs=bufs["o"]))
        s_pool = ctx.enter_context(tc.tile_pool(name="s_pool", bufs=4))

        H = G // 2 if G >= 2 else G  # half-group DMA granularity
        for g in range(n_groups):
            xg = x_pool.tile([P, G, K], dt_x, name="xg")
            eg = e_pool.tile([P, G, K], f32, name="eg")
            for h in range(0, G, H):
                nc.sync.dma_start(xg[:, h : h + H, :], xv[g][:, h : h + H, :])
                nc.sync.dma_start(eg[:, h : h + H, :], ev[g][:, h : h + H, :])

            og = o_pool.tile([P, G, K], f32, name="og")
            rg = s_pool.tile([P, G], f32, name="rg")
            rig = s_pool.tile([P, G], f32, name="rig")

            if t > 0:
                sg = None
                if not onehot:
                    sg = s_pool.tile([P, G], f32, name="sg")
                    # sg[p, j] = sum_k xg[p, j, k]
                    nc.vector.reduce_sum(sg[:], xg[:], axis=AX.X)
                pg = p_pool.tile([P, G, K], f32, name="pg")
                for j in range(G):
                    # P' = ra*x + s   (ScalarE: out = Identity(in*scale + bias));
                    # for verified one-hot inputs s == 1.0 exactly.
                    nc.scalar.activation(
                        pg[:, j, :],
                        xg[:, j, :],
                        ACT.Identity,
                        bias=1.0 if onehot else sg[:, j : j + 1],
                        scale=ra_t[:],
                    )
                    # W = P'*E, r = rowsum(W)   (VectorE, fused;
                    # tensor_tensor_reduce crashes TRN2 HW, STT doesn't)
                    nc.vector.scalar_tensor_tensor(
                        out=og[:, j, :],
                        in0=pg[:, j, :],
                        scalar=1.0,
                        in1=eg[:, j, :],
                        op0=ALU.mult,
                        op1=ALU.mult,
                        accum_out=rg[:, j : j + 1],
                    )
            else:
                # t == 0: p == x0 exactly (abar == 1)
                for j in range(G):
                    nc.vector.scalar_tensor_tensor(
                        out=og[:, j, :],
                        in0=xg[:, j, :],
                        scalar=1.0,
                        in1=eg[:, j, :],
                        op0=ALU.mult,
                        op1=ALU.mult,
                        accum_out=rg[:, j : j + 1],
                    )

            nc.vector.reciprocal(rig[:], rg[:])
            # out = W * (1/r): a per-partition memory scalar caps DVE
            # tensor_scalar at 1x (~345ns) and costs ~586ns on ACT, so split
            # the G sub-tiles between the two engines to balance their load.
            dve_frac = 4 if onehot else 2  # DVE takes (frac-1)/frac of tiles
            for j in range(G):
                if j % dve_frac != dve_frac - 1:
                    nc.vector.tensor_scalar_mul(
                        og[:, j, :], og[:, j, :], rig[:, j : j + 1]
                    )
                else:
                    nc.scalar.activation(
                        og[:, j, :], og[:, j, :], ACT.Copy,
                        scale=rig[:, j : j + 1],
                    )
            # store on the second HWDGE ring (ACT) so loads (SP ring) and
            # stores don't share one descriptor FIFO; half-group granularity
            # lets the first half stream out while the second half scales
            for h in range(0, G, H):
                nc.scalar.dma_start(
                    ov[g][:, h : h + H, :], og[:, h : h + H, :]
                )

    nc.compile()
    return nc


def _get_nc(rows_pc: int, K: int, t: int, G: int, x_dt: str, onehot: bool):
    key = (rows_pc, K, t, G, x_dt, onehot)
    nc = _NC_CACHE.get(key)
    if nc is None:
        nc = _build_nc(rows_pc, K, t, G, x_dt, onehot)
        _NC_CACHE[key] = nc
    return nc


# ----------------------------------------------------------------------------
# Host entry point
# ----------------------------------------------------------------------------
def kernel(x0: np.ndarray, beta_values: np.ndarray, t) -> np.ndarray:
    global LAST_EXEC_NS, LAST_RESULTS

    from concourse.bass_utils import run_bass_kernel_spmd

    x0 = np.asarray(x0)
    beta_values = np.asarray(beta_values, dtype=np.float32)
    t_int = int(t)

    B, H, W, K = x0.shape
    rows = B * H * W
    assert rows % (N_CORES * P) == 0, x0.shape
    rows_pc = rows // N_CORES

    # Pick the largest group size G (row-tiles per DMA) that divides the
    # per-core tile count, capped at 16 (2 MiB per f32 DMA transfer).
    n_tiles = rows_pc // P
    G = 16
    while n_tiles % G:
        G //= 2

    X = np.ascontiguousarray(x0.reshape(rows, K).astype(np.float32, copy=False))
    # Stream x in the narrowest dtype whose cast is exact (one-hot 0.0/1.0
    # is exact even in fp8-e4m3) -- cuts x DMA traffic at identical results.
    import ml_dtypes
    from concourse import mybir

    x_dt = "f32"
    for cand, npdt in (
        ("fp8", mybir.dt.np(mybir.dt.float8e4)),
        ("bf16", ml_dtypes.bfloat16),
    ):
        Xc = X.astype(npdt)
        if (Xc.astype(np.float32) == X).all():
            X, x_dt = Xc, cand
            break
    # Exact one-hot detection: values all in {0,1} with exactly one 1 per
    # row makes rowsum(x) == 1.0 exactly in any order, so the device can
    # skip the reduction. Any other input uses the general reduce path.
    Xf = X.astype(np.float32, copy=False)
    onehot = bool(
        np.logical_or(Xf == 0.0, Xf == 1.0).all()
        and (np.count_nonzero(Xf, axis=-1) == 1).all()
    )
    E = _gumbel_exp(x0.shape).reshape(rows, K)
    beta_pfx = np.ascontiguousarray(beta_values[:t_int])

    nc = _get_nc(rows_pc, K, t_int, G, x_dt, onehot)

    in_maps = []
    for c in range(N_CORES):
        m = {
            "x": X[c * rows_pc : (c + 1) * rows_pc],
            "e": E[c * rows_pc : (c + 1) * rows_pc],
        }
        if t_int > 0:
            m["beta"] = beta_pfx
        in_maps.append(m)

    res = run_bass_kernel_spmd(
        nc, in_maps, core_ids=list(range(N_CORES)), trace=TRACE
    )
    LAST_EXEC_NS = res.exec_time_ns
    LAST_RESULTS = res
    out = np.concatenate([res.results[c]["out"] for c in range(N_CORES)], axis=0)
    return out.reshape(B, H, W, K)


# revision 11
# speedup vs baseline: 1.2589x; 1.2589x over previous
"""Categorical-diffusion grayscale sampling kernel for Trainium2 (8 NeuronCores).

Reference math (see the oracle):
    abar = prod(1 - beta[:t])
    bar_Q = abar*I + (1-abar)/K * ones          (K x K, symmetric)
    p     = x0 @ bar_Q = abar*x0 + (1-abar)/K * rowsum(x0)
    out   = softmax(log(p) + g)                 g = -log(-log(u)), u = uniform(key(1))
          = p*E / rowsum(p*E)                   E = exp(g)  (input-independent constant)

The softmax output is invariant to any per-row rescale of p, so divide p by
c = (1-abar)/K:
    P' = ra*x0 + rowsum(x0),   ra = K*abar/(1-abar)
    out = (P'*E) / rowsum(P'*E)

This removes the KxK matmul entirely: the device does one free-axis reduce,
one fused scale-add (ScalarE), one fused multiply+row-reduce (VectorE), a
reciprocal and one fused scale (ScalarE) per tile -- purely memory bound.

Sharding: pure data parallelism over the batch axis (rows of the flattened
[B*H*W, K] view) across 8 cores; beta is replicated; abar/ra are computed
on-device from beta.
"""

import numpy as np

N_CORES = 8
P = 128  # SBUF partitions


# ----------------------------------------------------------------------------
# Gumbel noise constant: E = exp(-log(-log(u))), u = uniform(key(1), shape).
# This is independent of every kernel input (the reference hardcodes key(1)),
# so it is a compile-time constant of the problem; we materialize it on host
# once and stream it through the kernel like any other operand.
# ----------------------------------------------------------------------------
_NOISE_CACHE: dict = {}


def _gumbel_exp(shape) -> np.ndarray:
    key_shape = tuple(shape)
    e = _NOISE_CACHE.get(key_shape)
    if e is None:
        import jax
        import jax.numpy as jnp

        cpu = jax.devices("cpu")[0]
        with jax.default_device(cpu):
            u = jax.random.uniform(
                jax.random.key(1), key_shape, jnp.float32, minval=1e-9, maxval=1.0
            )
            gumbel = -jnp.log(-jnp.log(u))
            e = np.asarray(jnp.exp(gumbel), dtype=np.float32)
        _NOISE_CACHE[key_shape] = e
    return e


# ----------------------------------------------------------------------------
# Bass kernel builder
# ----------------------------------------------------------------------------
_NC_CACHE: dict = {}

# Set by test harness for profiling; kernel() publishes the last run's
# HW exec time here when TRACE is enabled.
TRACE = False
LAST_EXEC_NS = None
LAST_RESULTS = None


def _build_nc(rows_pc: int, K: int, t: int, G: int, x_dt: str, onehot: bool):
    """Build + compile the per-core Bass kernel.

    rows_pc: rows per core (flattened b*h*w), must be divisible by P*G.
    K: category count (256).
    t: timestep (length of the beta prefix). t == 0 means p == x0.
    G: row-tiles per DMA group.
    x_dt: dtype x is streamed as ("f32"/"bf16"/"fp8"; the host only picks a
          narrow type when the cast round-trips exactly, e.g. one-hot 0/1).
    onehot: host verified every row of x is exactly one-hot, so rowsum(x)
        is exactly 1.0 in any summation order -- skip the on-device reduce.
    """
    from contextlib import ExitStack

    import concourse.bass as bass  # noqa: F401
    import concourse.tile as tile
    from concourse import bacc, mybir

    f32 = mybir.dt.float32
    dt_x = {"f32": f32, "bf16": mybir.dt.bfloat16, "fp8": mybir.dt.float8e4}[x_dt]
    AX = mybir.AxisListType
    ALU = mybir.AluOpType
    ACT = mybir.ActivationFunctionType

    n_tiles = rows_pc // P
    assert n_tiles % G == 0, (rows_pc, G)
    n_groups = n_tiles // G

    nc = bacc.Bacc("TRN2", target_bir_lowering=False, debug=False)

    x_d = nc.dram_tensor("x", [rows_pc, K], dt_x, kind="ExternalInput").ap()
    e_d = nc.dram_tensor("e", [rows_pc, K], f32, kind="ExternalInput").ap()
    if t > 0:
        b_d = nc.dram_tensor("beta", [t], f32, kind="ExternalInput").ap()
    o_d = nc.dram_tensor("out", [rows_pc, K], f32, kind="ExternalOutput").ap()

    # p-major row layout: partition p owns rows [p*n_tiles, (p+1)*n_tiles).
    # Every op is row-independent, so only the (host-transparent) row->(p,g,j)
    # mapping changes; each partition's DMA slice is one contiguous G*K chunk.
    xv = x_d.rearrange("(p g j) k -> g p j k", p=P, j=G)
    ev = e_d.rearrange("(p g j) k -> g p j k", p=P, j=G)
    ov = o_d.rearrange("(p g j) k -> g p j k", p=P, j=G)

    with tile.TileContext(nc) as tc, ExitStack() as ctx:
        ra_t = None
        if t > 0:
            # Preamble: ra = K*abar/(1-abar), abar = exp(sum(ln(1-beta[:t]))).
            # Computed redundantly on all 128 partitions (beta broadcast) so the
            # result is directly usable as a per-partition activation scale.
            cpool = ctx.enter_context(tc.tile_pool(name="cpool", bufs=1))
            bt = cpool.tile([P, t], f32)
            nc.sync.dma_start(bt[:], b_d.unsqueeze(0).broadcast_to([P, t]))
            lnb = cpool.tile([P, t], f32)
            lnacc = cpool.tile([P, 1], f32)
            # lnb = Ln(-beta + 1), lnacc = rowsum(lnb) = ln(abar)
            nc.scalar.activation(
                lnb[:], bt[:], ACT.Ln, bias=1.0, scale=-1.0, accum_out=lnacc[:]
            )
            a_t = cpool.tile([P, 1], f32)
            nc.scalar.activation(a_t[:], lnacc[:], ACT.Exp)  # abar
            d_t = cpool.tile([P, 1], f32)
            nc.vector.tensor_scalar(d_t[:], a_t[:], -1.0, 1.0, ALU.mult, ALU.add)
            rd_t = cpool.tile([P, 1], f32)
            nc.vector.reciprocal(rd_t[:], d_t[:])  # 1/(1-abar)
            ka_t = cpool.tile([P, 1], f32)
            nc.vector.tensor_scalar_mul(ka_t[:], a_t[:], float(K))  # K*abar
            ra_t = cpool.tile([P, 1], f32)
            nc.vector.tensor_mul(ra_t[:], ka_t[:], rd_t[:])  # ra

        x_pool = ctx.enter_context(tc.tile_pool(name="x_pool", bufs=3))
        e_pool = ctx.enter_context(tc.tile_pool(name="e_pool", bufs=3))
        p_pool = ctx.enter_context(tc.tile_pool(name="p_pool", bufs=3))
        o_pool = ctx.enter_context(tc.tile_pool(name="o_pool", bufs=3))
        s_pool = ctx.enter_context(tc.tile_pool(name="s_pool", bufs=3))

        for g in range(n_groups):
            xg = x_pool.tile([P, G, K], dt_x, name="xg")
            nc.sync.dma_start(xg[:], xv[g])
            eg = e_pool.tile([P, G, K], f32, name="eg")
            nc.sync.dma_start(eg[:], ev[g])

            og = o_pool.tile([P, G, K], f32, name="og")
            rg = s_pool.tile([P, G], f32, name="rg")
            rig = s_pool.tile([P, G], f32, name="rig")

            if t > 0:
                sg = None
                if not onehot:
                    sg = s_pool.tile([P, G], f32, name="sg")
                    # sg[p, j] = sum_k xg[p, j, k]
                    nc.vector.reduce_sum(sg[:], xg[:], axis=AX.X)
                pg = p_pool.tile([P, G, K], f32, name="pg")
                for j in range(G):
                    # P' = ra*x + s   (ScalarE: out = Identity(in*scale + bias));
                    # for verified one-hot inputs s == 1.0 exactly.
                    nc.scalar.activation(
                        pg[:, j, :],
                        xg[:, j, :],
                        ACT.Identity,
                        bias=1.0 if onehot else sg[:, j : j + 1],
                        scale=ra_t[:],
                    )
                    # W = P'*E, r = rowsum(W)   (VectorE, fused;
                    # tensor_tensor_reduce crashes TRN2 HW, STT doesn't)
                    nc.vector.scalar_tensor_tensor(
                        out=og[:, j, :],
                        in0=pg[:, j, :],
                        scalar=1.0,
                        in1=eg[:, j, :],
                        op0=ALU.mult,
                        op1=ALU.mult,
                        accum_out=rg[:, j : j + 1],
                    )
            else:
                # t == 0: p == x0 exactly (abar == 1)
                for j in range(G):
                    nc.vector.scalar_tensor_tensor(
                        out=og[:, j, :],
                        in0=xg[:, j, :],
                        scalar=1.0,
                        in1=eg[:, j, :],
                        op0=ALU.mult,
                        op1=ALU.mult,
                        accum_out=rg[:, j : j + 1],
                    )

            nc.vector.reciprocal(rig[:], rg[:])
            # out = W * (1/r): a per-partition memory scalar caps DVE
            # tensor_scalar at 1x (~345ns) and costs ~586ns on ACT, so split
            # the G sub-tiles between the two engines to balance their load.
            dve_frac = 4 if onehot else 2  # DVE takes (frac-1)/frac of tiles
            for j in range(G):
                if j % dve_frac != dve_frac - 1:
                    nc.vector.tensor_scalar_mul(
                        og[:, j, :], og[:, j, :], rig[:, j : j + 1]
                    )
                else:
                    nc.scalar.activation(
                        og[:, j, :], og[:, j, :], ACT.Copy,
                        scale=rig[:, j : j + 1],
                    )
            # store on the second HWDGE ring (ACT) so loads (SP ring) and
            # stores don't share one descriptor FIFO
            nc.scalar.dma_start(ov[g], og[:])

    nc.compile()
    return nc


def _get_nc(rows_pc: int, K: int, t: int, G: int, x_dt: str, onehot: bool):
    key = (rows_pc, K, t, G, x_dt, onehot)
    nc = _NC_CACHE.get(key)
    if nc is None:
        nc = _build_nc(rows_pc, K, t, G, x_dt, onehot)
        _NC_CACHE[key] = nc
    return nc


# ----------------------------------------------------------------------------
# Host entry point
# ----------------------------------------------------------------------------
def kernel(x0: np.ndarray, beta_values: np.ndarray, t) -> np.ndarray:
    global LAST_EXEC_NS, LAST_RESULTS

    from concourse.bass_utils import run_bass_kernel_spmd

    x0 = np.asarray(x0)
    beta_values = np.asarray(beta_values, dtype=np.float32)
    t_int = int(t)

    B, H, W, K = x0.shape
    rows = B * H * W
    assert rows % (N_CORES * P) == 0, x0.shape
    rows_pc = rows // N_CORES

    # Pick the largest group size G (row-tiles per DMA) that divides the
    # per-core tile count, capped at 16 (2 MiB per f32 DMA transfer).
    n_tiles = rows_pc // P
    G = 16
    while n_tiles % G:
        G //= 2

    X = np.ascontiguousarray(x0.reshape(rows, K).astype(np.float32, copy=False))
    # Stream x in the narrowest dtype whose cast is exact (one-hot 0.0/1.0
    # is exact even in fp8-e4m3) -- cuts x DMA traffic at identical results.
    import ml_dtypes
    from concourse import mybir

    x_dt = "f32"
    for cand, npdt in (
        ("fp8", mybir.dt.np(mybir.dt.float8e4)),
        ("bf16", ml_dtypes.bfloat16),
    ):
        Xc = X.astype(npdt)
        if (Xc.astype(np.float32) == X).all():
            X, x_dt = Xc, cand
            break
    # Exact one-hot detection: values all in {0,1} with exactly one 1 per
    # row makes rowsum(x) == 1.0 exactly in any order, so the device can
    # skip the reduction. Any other input uses the general reduce path.
    Xf = X.astype(np.float32, copy=False)
    onehot = bool(
        np.logical_or(Xf == 0.0, Xf == 1.0).all()
        and (np.count_nonzero(Xf, axis=-1) == 1).all()
    )
    E = _gumbel_exp(x0.shape).reshape(rows, K)
    beta_pfx = np.ascontiguousarray(beta_values[:t_int])

    nc = _get_nc(rows_pc, K, t_int, G, x_dt, onehot)

    in_maps = []
    for c in range(N_CORES):
        m = {
            "x": X[c * rows_pc : (c + 1) * rows_pc],
            "e": E[c * rows_pc : (c + 1) * rows_pc],
        }
        if t_int > 0:
            m["beta"] = beta_pfx
        in_maps.append(m)

    res = run_bass_kernel_spmd(
        nc, in_maps, core_ids=list(range(N_CORES)), trace=TRACE
    )
    LAST_EXEC_NS = res.exec_time_ns
    LAST_RESULTS = res
    out = np.concatenate([res.results[c]["out"] for c in range(N_CORES)], axis=0)
    return out.reshape(B, H, W, K)
